# revision 37
# baseline (speedup 1.0000x reference)
"""DAHHConv (hypergraph conv) Trainium2 Bass kernel, 8-core SPMD.

Math (reference):
    x' = x @ theta                      # [B,N,C]
    xe = (H^T x') / deg_e               # [B,E,C], deg_e = sum_n H
    xn = (H xe) / deg_n                 # [B,N,C], deg_n = sum_e H
    out = xn + bias

Sharding: 8 cores = 4 batches x 2 e-halves; core c -> batch b=c//2,
half h=c%2. Both phases shard the HYPEREDGE dim: core (b,h) owns
e in [1024h, 1024h+1024).
  Phase 1 (edge aggregation, contract n): me[65,1024] = x_aug^T @ H_n
  over ALL N for the own e-half - fully local.
  Phase 3 (node aggregation, contract e): each core produces the
  PARTIAL y^T[65, 8192] = xe_aug^T @ H_e^T over its own e-half for the
  FULL node range. The pair-sum over the two e-halves and the deg_n
  division happen in the host-side unshard (partial-sum gather), so the
  kernel needs NO inter-core collective: measured here, the ncfw
  AllGather costs 40-60us wall (entry barrier + mesh starved behind the
  kernel's own DMA flood), dwarfing the 133KB payload.

Key structure:
  - No separate x@theta phase: the host supplies x_aug chunks (with a
    baked ones-column) already in [128n, 65] stationary layout; theta is
    applied AFTER the n-contraction on the small me_raw[65,1024] via a
    block-diagonal th_aug (2 matmuls), so deg_e (row 64) survives.
  - Row 64 of the x_aug/xe_aug stationaries makes deg_e / deg_n fall
    out of the same matmul streams for free.
  - ht is host-packed n-span-major: tile s = [128e x (8 chunks x 1024n)]
    so phase-3 span s needs only tile s (pipelined arrival).
  - hn tile 0's DMA is quarter-split so the first matmul starts early.
  - DMA FIFO split: bulk loads on nc.sync (HWDGE/SP), output stores on
    nc.scalar (HWDGE/ACT) to avoid head-of-line blocking.
"""

import numpy as np
import ml_dtypes

B, N, E, C = 4, 8192, 2048, 64
NCORES = 8
EH = E // 2          # 1024: e-range per core
CA = C + 1           # 65: feature dim augmented with ones/deg column
NCHUNK = N // 128    # 64 n-chunks in phase 1
HNTILES = N // 512   # 16 hn DMA tiles (512 rows each)
ECHUNK = EH // 128   # 8 e-chunks in phase 3 (own half only)
NSPAN = 1024         # phase-3 output span (2 PSUM banks at fp32)
NSPANS = N // NSPAN  # 8 spans covering the FULL node range
BF16 = ml_dtypes.bfloat16
FP8 = ml_dtypes.float8_e4m3

_cache = {}


def _split_waits_json(raw: bytes) -> bytes:
    """BIR post-pass: this walrus/ISA build allows only ONE sync wait per
    instruction, but the Tile scheduler attaches several. Hoist all but
    the last wait of each instruction onto standalone EventSemaphore
    instructions inserted just before it on the same engine (waits are
    pure preconditions, so running them earlier on the same engine
    stream is equivalent)."""
    import json

    m = json.loads(raw)
    ctr = 0
    for f in m["functions"]:
        for blk in f["blocks"]:
            new = []
            for inst in blk["instructions"]:
                si = inst.get("sync_info")
                waits = (si or {}).get("on_wait") or []
                if len(waits) > 1:
                    for w in waits[:-1]:
                        ctr += 1
                        new.append(
                            {
                                "debug": inst.get("debug", 0),
                                "engine": inst["engine"],
                                "ins": [],
                                "name": f"{inst['name']}-xw{ctr}",
                                "opcode": "EventSemaphore",
                                "outs": [],
                                "sync_info": {"on_update": [], "on_wait": [w]},
                            }
                        )
                    si["on_wait"] = [waits[-1]]
                new.append(inst)
            blk["instructions"] = new
    return json.dumps(m).encode()


def build_bass():
    import concourse.bass as bass
    import concourse.mybir as mybir
    from concourse.tile import TileContext
    from concourse import masks

    dt = mybir.dt
    nc = bass.Bass()

    hn = nc.declare_dram_parameter("hn", [N, EH], dt.float8e4, isOutput=False)
    # n-span-major: row-block s = [128, 8*1024] covering n in
    # [1024s, 1024s+1024) for all 8 own-e chunks
    ht = nc.declare_dram_parameter("ht", [NSPANS * 128, ECHUNK * NSPAN],
                                   dt.float8e4, isOutput=False)
    xp = nc.declare_dram_parameter("xp", [128, NCHUNK * CA], dt.bfloat16, isOutput=False)
    th = nc.declare_dram_parameter("th", [CA, CA], dt.bfloat16, isOutput=False)
    # PARTIAL y^T for the full node range. Host sums the pair and
    # divides by deg_n (partial-sum unshard).
    out = nc.declare_dram_parameter("out", [C, N], dt.bfloat16, isOutput=True)

    with TileContext(nc) as tc:
        with (
            tc.tile_pool(name="const", bufs=1) as const,
            tc.tile_pool(name="persist", bufs=1) as persist,
            tc.tile_pool(name="hn_pool", bufs=16) as hn_pool,
            tc.tile_pool(name="ht_pool", bufs=1) as ht_pool,
            tc.tile_pool(name="small", bufs=2) as small,
            tc.tile_pool(name="opool", bufs=6) as opool,
        ):
            ident = const.tile([128, 128], dt.float32)
            masks.make_identity(nc, ident[:])
            th_sb = const.tile([CA, CA], dt.bfloat16)
            nc.sync.dma_start(th_sb[:], th[:])
            # x_aug chunks, host-packed: chunk j at cols [65j, 65j+65)
            xp_sb = persist.tile([128, NCHUNK * CA], dt.bfloat16)
            XQ = NCHUNK * CA // 4
            nc.sync.dma_start(xp_sb[:, 0:XQ], xp[:, 0:XQ])

            ht_tiles = [
                ht_pool.tile([128, ECHUNK * NSPAN], dt.float8e4,
                             tag=f"ht{s}", name=f"ht{s}")
                for s in range(NSPANS)
            ]

            # xe_aug[e,65] chunks; col 64 = 1 (set once; per-chunk writes
            # only touch cols 0:64 so the partial deg_n stays exact)
            xe_sb = persist.tile([128, ECHUNK * CA], dt.bfloat16)
            xe_v = xe_sb[:].rearrange("p (c w) -> p c w", w=CA)
            nc.vector.memset(xe_v[:, :, C : C + 1], 1.0)

            # ---- phase 1: me_raw^T[65,1024] = x_aug^T @ H_n  (accum) ----
            # hn tile t covers DRAM rows [512t, 512t+512): partition p
            # holds rows 512t+4p..512t+4p+3 (4KB contiguous lines); the
            # matching x_aug chunks are j = 4t..4t+3 (xp host-permuted).
            with tc.tile_pool(name="psA", bufs=1, space="PSUM") as psA:
                ps_me = psA.tile([CA, EH], dt.float32, tag="me")
                for t in range(HNTILES):
                    hn_t = hn_pool.tile([128, 4 * EH], dt.float8e4, tag="hn")
                    src = hn[512 * t : 512 * (t + 1), :].rearrange(
                        "(p four) e -> p (four e)", four=4
                    )
                    if t == 0:
                        # quarter-split so the first matmul starts early
                        for q in range(4):
                            nc.sync.dma_start(
                                hn_t[:, EH * q : EH * (q + 1)],
                                src[:, EH * q : EH * (q + 1)],
                            )
                    else:
                        nc.sync.dma_start(hn_t[:], src)
                    if t <= 2:
                        q = t + 1
                        nc.sync.dma_start(
                            xp_sb[:, XQ * q : XQ * (q + 1)],
                            xp[:, XQ * q : XQ * (q + 1)],
                        )
                    for q in range(4):
                        j = 4 * t + q
                        for half in range(2):
                            nc.tensor.matmul(
                                ps_me[:, 512 * half : 512 * (half + 1)],
                                xp_sb[:, CA * j : CA * (j + 1)],
                                hn_t[:, 1024 * q + 512 * half : 1024 * q + 512 * (half + 1)],
                                start=(t == 0 and q == 0),
                                stop=(t == HNTILES - 1 and q == 3),
                            )
                me_raw = persist.tile([CA, EH], dt.bfloat16)
                for half in range(2):
                    nc.vector.tensor_copy(
                        me_raw[:, 512 * half : 512 * (half + 1)],
                        ps_me[:, 512 * half : 512 * (half + 1)],
                    )

            # ht span-tiles stream right after the hn flood (sync FIFO);
            # span s needs only tile s -> pipelined phase-3 start
            for s in range(NSPANS):
                nc.sync.dma_start(ht_tiles[s][:],
                                  ht[128 * s : 128 * (s + 1), :])

            # ---- theta on the e-side: me = th_aug^T @ me_raw ----
            # (block-diagonal th_aug keeps row 64 = deg_e)
            me_f32 = persist.tile([CA, EH], dt.float32)
            with tc.tile_pool(name="psB", bufs=1, space="PSUM") as psB:
                ps_me2 = psB.tile([CA, EH], dt.float32, tag="me2")
                for half in range(2):
                    nc.tensor.matmul(
                        ps_me2[:, 512 * half : 512 * (half + 1)],
                        th_sb[:],
                        me_raw[:, 512 * half : 512 * (half + 1)],
                        start=True,
                        stop=True,
                    )
                    nc.vector.tensor_copy(
                        me_f32[:, 512 * half : 512 * (half + 1)],
                        ps_me2[:, 512 * half : 512 * (half + 1)],
                    )

            with (
                tc.tile_pool(name="psT", bufs=2, space="PSUM") as psT,
                tc.tile_pool(name="psY", bufs=3, space="PSUM") as psY,
            ):
                # ---- phase 2: xe_aug chunks = (me/deg_e)^T ----
                for k in range(ECHUNK):
                    ps_tr = psT.tile([128, CA], dt.float32, tag="tr")
                    nc.tensor.transpose(
                        ps_tr[:], me_f32[:, 128 * k : 128 * (k + 1)],
                        ident[0:CA, 0:CA],
                    )
                    rec = small.tile([128, 1], dt.float32, tag="rec")
                    nc.vector.reciprocal(rec[:], ps_tr[:, C : C + 1])
                    nc.vector.tensor_scalar_mul(
                        xe_v[:, k, 0:C], ps_tr[:, 0:C], rec[:]
                    )

                # ---- phase 3: partial y^T spans over the full N ----
                # Dual-stream column tiling: the M=64 feature stationary
                # only needs col-groups 0-1, so a second concurrent
                # matmul stream runs on col-groups 2-3 (tile_position
                # (0,64), output partitions 64-127). Each stream
                # accumulates in its OWN bank (start=True clears a whole
                # bank's has_written bits, so sharing one would race).
                for s in range(NSPANS):
                    ps_a = psY.tile([64, 512], dt.float32, tag="yA",
                                    name=f"yA{s}")
                    ps_b = psY.tile([128, 512], dt.float32, tag="yB",
                                    name=f"yB{s}")
                    for k in range(ECHUNK):
                        nc.tensor.matmul(
                            ps_a[:],
                            xe_sb[:, CA * k : CA * k + C],
                            ht_tiles[s][:, NSPAN * k : NSPAN * k + 512],
                            start=(k == 0),
                            stop=(k == ECHUNK - 1),
                            tile_position=(0, 0),
                        )
                        nc.tensor.matmul(
                            ps_b[64:128, :],
                            xe_sb[:, CA * k : CA * k + C],
                            ht_tiles[s][:, NSPAN * k + 512 : NSPAN * (k + 1)],
                            start=(k == 0),
                            stop=(k == ECHUNK - 1),
                            tile_position=(0, 64),
                        )
                    o_sb = opool.tile([128, 512], dt.bfloat16, tag="o_sb")
                    nc.vector.tensor_copy(o_sb[0:64, :], ps_a[:])
                    nc.vector.tensor_copy(o_sb[64:128, :], ps_b[64:128, :])
                    # one full-128-partition store: partition two*64+c ->
                    # DRAM [c, s*1024 + two*512 + f] (manual AP; the
                    # rearrange helper rejects this grouping)
                    dst = bass.AP(
                        tensor=out,
                        offset=s * NSPAN,
                        ap=[[512, 2], [N, 64], [1, 512]],
                    )
                    nc.scalar.dma_start(dst, o_sb[:])

    orig_to_json = nc.to_json_bytes
    nc.to_json_bytes = lambda: _split_waits_json(orig_to_json())
    return nc


def _fp8_exact(a):
    # H is 0/1: 1.0 is exactly 0x38 in float8_e4m3.
    return (np.where(a != 0, 0x38, 0)).astype(np.uint8).view(FP8)


def _prepare_in_maps(x, H, theta):
    x = np.ascontiguousarray(x, dtype=np.float32)
    H = np.ascontiguousarray(H, dtype=np.float32)
    th16 = np.zeros((CA, CA), dtype=np.float32)
    th16[0:C, 0:C] = np.asarray(theta, dtype=np.float32)
    th16[C, C] = 1.0
    th16 = th16.astype(BF16)
    _cache["rdeg_n"] = 1.0 / H.sum(axis=2)          # [B, N] for _assemble
    in_maps = []
    for c in range(NCORES):
        b, h = divmod(c, 2)
        own = H[b, :, EH * h : EH * (h + 1)]            # [N, EH]
        hnc = _fp8_exact(np.ascontiguousarray(own))
        # ht n-span-major: [s, p, k, n'] = own[1024s+n', 128k+p]
        t4 = own.reshape(NSPANS, NSPAN, ECHUNK, 128)
        htc = _fp8_exact(np.ascontiguousarray(
            t4.transpose(0, 3, 2, 1).reshape(NSPANS * 128, ECHUNK * NSPAN)
        ))
        # phase-1 consumes n in blocks of 512 as [128 partitions x 4 rows]:
        # chunk j = 4t+q, partition p <-> DRAM row 512t+4p+q. The host
        # packs x_aug into the exact SBUF stationary layout.
        xa = np.concatenate(
            [x[b], np.ones((N, 1), dtype=np.float32)], axis=1
        ).astype(BF16)                                   # [N, 65]
        xr = xa.reshape(HNTILES, 128, 4, CA)
        xpc = np.ascontiguousarray(
            xr.transpose(1, 0, 2, 3).reshape(128, NCHUNK * CA)
        )
        in_maps.append({"hn": hnc, "ht": htc, "xp": xpc, "th": th16})
    return in_maps


def _assemble(results, bias):
    # partial-sum unshard: sum the pair's e-half contributions, divide
    # by deg_n (stashed by _prepare_in_maps), transpose, add bias
    rdeg = _cache["rdeg_n"]
    out = np.empty((B, N, C), dtype=np.float32)
    for b in range(B):
        r = (results[2 * b]["out"].astype(np.float32)
             + results[2 * b + 1]["out"].astype(np.float32))  # [C, N]
        out[b] = (r * rdeg[b][None, :]).T
    out += np.asarray(bias, dtype=np.float32)[None, None, :]
    return out


def get_nc():
    if "nc" not in _cache:
        _cache["nc"] = build_bass()
    return _cache["nc"]


def kernel(x, H, theta, bias):
    from concourse.bass_utils import run_bass_kernel_spmd

    nc = get_nc()
    in_maps = _prepare_in_maps(x, H, theta)
    res = run_bass_kernel_spmd(nc, in_maps, list(range(NCORES)))
    return _assemble(res.results, bias)
